# revision 6
# baseline (speedup 1.0000x reference)
"""KoLeo loss kernel for Trainium2 (8 NeuronCores, Bass/Tile).

Math: reference normalizes rows of student_output [8192, 384], finds each
row's nearest neighbor by cosine similarity (self masked), and returns
  loss = -mean(log(||x_i - x_nn|| + eps)).
For unit vectors ||x_i - x_j||^2 = 2 - 2*dot(x_i, x_j), so only the max
off-diagonal dot per row is needed -- no argmax indices, no gather.

Sharding: core m gets x rotated by m*1024 rows (bf16). It normalizes on
device, transposes via DMA-xbar round trip to get x^T in SBUF, computes its
[1024, 8192] similarity block (bf16 matmul, fp32 PSUM), masks the diagonal
(which the rotation pins to columns [mt*128, mt*128+128) of m-tile mt), and
row-max-reduces split across DVE (direct fp32 reduce from PSUM) and
ACT->bf16-convert + DVE tensor_max (2x mode). Host combines the 8 per-core
[128, 8] max-dot tiles into the scalar loss.
"""

import os
import numpy as np
import ml_dtypes

import concourse.bass as bass
import concourse.tile as tile
from concourse import bacc, mybir
from concourse.bass_utils import run_bass_kernel_spmd

F32 = mybir.dt.float32
BF16 = mybir.dt.bfloat16
AX = mybir.AxisListType
OP = mybir.AluOpType
AF = mybir.ActivationFunctionType

N, D = 8192, 384
P = 128
NCORES = 8
KT = D // P            # 3 contraction tiles
RG = 8                 # row groups of 1024
CPG = 8                # chunks of 128 rows per group
MT = 8                 # m-tiles: own block = first 1024 rotated rows
NF = 512               # matmul moving free dim (one PSUM bank)
NT = N // NF           # 16 n-tiles
MASKVAL = -4.0         # diag additive mask; masked value ~ -3 < any cosine

# Per-(mt) reduce-path assignment for the 16 n-tiles:
#   'A'  = DVE reduce_max direct from PSUM (fp32, 1x)
#   'B'  = ACT copy->bf16 SBUF, DVE tensor_max accumulate (2x)
# diag tiles (nt 0..1) must be 'B' (mask applied on the SBUF copy).
PATHS = ['B', 'B', 'A', 'A', 'A', 'B', 'A', 'B',
         'A', 'B', 'A', 'B', 'A', 'B', 'A', 'B']
NA = PATHS.count('A')

_CACHE = {}


def _build_program():
    nc = bacc.Bacc("TRN2", target_bir_lowering=False, debug=False,
                   num_devices=NCORES)
    x_in = nc.dram_tensor("x", [N, D], BF16, kind="ExternalInput").ap()
    negid_in = nc.dram_tensor("negid", [P, P], BF16, kind="ExternalInput").ap()
    md_out = nc.dram_tensor("maxdot", [P, MT], F32, kind="ExternalOutput").ap()

    with tile.TileContext(nc) as tc:
        with (
            tc.tile_pool(name="consts", bufs=1) as const_pool,
            tc.tile_pool(name="xin", bufs=3) as xin_pool,
            tc.tile_pool(name="xnorm", bufs=3) as xn_pool,
            tc.tile_pool(name="stats", bufs=1) as stat_pool,
            tc.tile_pool(name="small", bufs=2) as small_pool,
            tc.tile_pool(name="sq", bufs=2) as sq_pool,
            tc.tile_pool(name="xt", bufs=1) as xt_pool,
            tc.tile_pool(name="xndram", bufs=1, space="DRAM") as dram_pool,
            tc.tile_pool(name="psum", bufs=8, space="PSUM") as psum_pool,
            tc.tile_pool(name="conv", bufs=6) as conv_pool,
            tc.tile_pool(name="accb", bufs=2) as accb_pool,
            tc.tile_pool(name="parts", bufs=2) as part_pool,
            tc.tile_pool(name="outp", bufs=1) as out_pool,
        ):
            negid = const_pool.tile([P, P], BF16)
            nc.sync.dma_start(negid, negid_in)

            ss_all = stat_pool.tile([P, RG * CPG], F32)   # row sum-of-squares
            rn_all = stat_pool.tile([P, RG * CPG], F32)   # 1/row-norm
            xn_dram = dram_pool.tile([N, D], BF16)
            xT = []
            for k in range(KT):
                xTk = xt_pool.tile([P, N], BF16, name=f"xT{k}", tag=f"xT{k}")
                xT.append(xTk)

            # ---- Phase 1: load, row norms, scale, store normalized bf16 ----
            x_view = x_in.rearrange("(g s p) d -> g p s d", p=P, s=CPG)
            xn_view = xn_dram.rearrange("(g s p) d -> g p s d", p=P, s=CPG)
            for g in range(RG):
                xg = xin_pool.tile([P, CPG, D], BF16, tag="xg")
                nc.sync.dma_start(xg, x_view[g])
                for s in range(CPG):
                    c = g * CPG + s
                    sq = sq_pool.tile([P, D], BF16, tag="sq")
                    nc.scalar.activation(sq, xg[:, s], AF.Square,
                                         accum_out=ss_all[:, c:c + 1])
                # rnorm = rsqrt(ss): recip -> sqrt -> 2 Newton steps
                sl = ss_all[:, g * CPG:(g + 1) * CPG]
                rn = rn_all[:, g * CPG:(g + 1) * CPG]
                rec = small_pool.tile([P, CPG], F32, tag="rec")
                nc.vector.reciprocal(rec, sl)
                nc.scalar.activation(rn, rec, AF.Sqrt)
                for _ in range(2):
                    t1 = small_pool.tile([P, CPG], F32, tag="t1")
                    nc.vector.tensor_mul(t1, rn, rn)
                    nc.vector.tensor_mul(t1, t1, sl)
                    nc.vector.tensor_scalar(t1, t1, -0.5, 1.5, OP.mult, OP.add)
                    nc.vector.tensor_mul(rn, rn, t1)
                xng = xn_pool.tile([P, CPG, D], BF16, tag="xng")
                for s in range(CPG):
                    c = g * CPG + s
                    nc.vector.tensor_scalar_mul(
                        xng[:, s], xg[:, s], rn_all[:, c:c + 1])
                nc.gpsimd.dma_start(xn_view[g], xng)
                # ---- Phase 2 (interleaved): transpose this group back in ----
                for k in range(KT):
                    nc.scalar.dma_start_transpose(
                        xT[k][:, g * 1024:(g + 1) * 1024],
                        xn_dram[g * 1024:(g + 1) * 1024, k * P:(k + 1) * P])

            # ---- Phase 3: block matmul + row-max reduce ----
            md_sb = out_pool.tile([P, MT], F32)
            for mt in range(MT):
                parts = part_pool.tile([P, NA + 1], F32, tag="parts")
                accb = accb_pool.tile([P, NF], BF16, tag="accb")
                nc.gpsimd.memset(accb, -3.0)
                ia = 0
                diag_nt = (mt * P) // NF
                for nt in range(NT):
                    ps = psum_pool.tile([P, NF], F32, tag="ps")
                    for k in range(KT):
                        nc.tensor.matmul(
                            ps,
                            xT[k][:, mt * P:(mt + 1) * P],
                            xT[k][:, nt * NF:(nt + 1) * NF],
                            start=(k == 0), stop=(k == KT - 1))
                    if PATHS[nt] == 'A' and nt != diag_nt:
                        nc.vector.reduce_max(
                            parts[:, ia:ia + 1], ps, axis=AX.X)
                        ia += 1
                    else:
                        cv = conv_pool.tile([P, NF], BF16, tag="cv")
                        nc.scalar.copy(cv, ps)
                        if nt == diag_nt:
                            o = (mt * P) % NF
                            nc.vector.tensor_add(
                                cv[:, o:o + P], cv[:, o:o + P], negid)
                        nc.vector.tensor_max(accb, accb, cv)
                assert ia == NA or ia == NA - 1
                # pad unused A slots so the final reduce is well-defined
                while ia < NA:
                    nc.vector.tensor_copy(parts[:, ia:ia + 1], parts[:, 0:1])
                    ia += 1
                nc.vector.reduce_max(parts[:, NA:NA + 1], accb, axis=AX.X)
                nc.vector.reduce_max(md_sb[:, mt:mt + 1], parts, axis=AX.X)
            nc.sync.dma_start(md_out, md_sb)

    nc.compile()
    return nc


def _get_program():
    if "nc" not in _CACHE:
        _CACHE["nc"] = _build_program()
    return _CACHE["nc"]


def _make_in_maps(student_output: np.ndarray):
    x = np.asarray(student_output, dtype=np.float32)
    assert x.shape == (N, D)
    negid = (MASKVAL * np.eye(P, dtype=np.float32)).astype(ml_dtypes.bfloat16)
    in_maps = []
    for m in range(NCORES):
        xr = np.roll(x, -1024 * m, axis=0).astype(ml_dtypes.bfloat16)
        in_maps.append({"x": xr, "negid": negid})
    return in_maps


def _combine(results) -> np.float32:
    md = np.empty(N, dtype=np.float64)
    for m in range(NCORES):
        blk = np.asarray(results[m]["maxdot"], dtype=np.float64)  # [P, MT]
        md[m * 1024:(m + 1) * 1024] = blk.T.reshape(-1)
    d2 = np.maximum(2.0 - 2.0 * md, 0.0)
    d = np.sqrt(d2)
    loss = -np.mean(np.log(d + 1e-8))
    return np.float32(loss)


def run(student_output: np.ndarray, trace: bool = False):
    nc = _get_program()
    in_maps = _make_in_maps(student_output)
    res = run_bass_kernel_spmd(nc, in_maps, core_ids=list(range(NCORES)),
                               trace=trace)
    return _combine(res.results), res


def kernel(student_output: np.ndarray) -> np.ndarray:
    out, _ = run(student_output,
                 trace=bool(int(os.environ.get("KOLEO_TRACE", "0"))))
    return out


# revision 9
# speedup vs baseline: 1.2113x; 1.2113x over previous
"""KoLeo loss kernel for Trainium2 (8 NeuronCores, Bass/Tile).

Math: reference normalizes rows of student_output [8192, 384], finds each
row's nearest neighbor by cosine similarity (self masked), and returns
  loss = -mean(log(||x_i - x_nn|| + eps)).
For unit vectors ||x_i - x_j||^2 = 2 - 2*dot(x_i, x_j), so only the max
off-diagonal dot per row is needed -- no argmax indices, no gather.

Sharding: core m gets x rotated by m*1024 rows (bf16). It normalizes on
device, transposes via DMA-xbar round trip to get x^T in SBUF, computes its
[1024, 8192] similarity block (bf16 matmul, fp32 PSUM), masks the diagonal
(which the rotation pins to columns [mt*128, mt*128+128) of m-tile mt), and
row-max-reduces split across DVE (direct fp32 reduce from PSUM) and
ACT->bf16-convert + DVE tensor_max (2x mode). Host combines the 8 per-core
[128, 8] max-dot tiles into the scalar loss.
"""

import os
import numpy as np
import ml_dtypes

import concourse.bass as bass
import concourse.tile as tile
from concourse import bacc, mybir
from concourse.bass_utils import run_bass_kernel_spmd

F32 = mybir.dt.float32
BF16 = mybir.dt.bfloat16
AX = mybir.AxisListType
OP = mybir.AluOpType
AF = mybir.ActivationFunctionType

N, D = 8192, 384
P = 128
NCORES = 8
KT = D // P            # 3 contraction tiles
RG = 8                 # row groups of 1024
CPG = 8                # chunks of 128 rows per group
MT = 8                 # m-tiles: own block = first 1024 rotated rows
NF = 512               # matmul moving free dim (one PSUM bank)
NT = N // NF           # 16 n-tiles
MASKVAL = -4.0         # diag additive mask; masked value ~ -3 < any cosine

# Reduce-path per (group, mt) unit (one [128, 1024] PSUM pair):
#   'A' = DVE reduce_max direct from PSUM (fp32, 1x)
#   'B' = ACT copy->bf16 SBUF, DVE tensor_max accumulate (2x)
# group 0 units must be 'B' (diag mask applied on the SBUF copy).
def _is_a(g, mt):
    return g in (2, 4, 6) or (g == 7 and mt < 3)

_CACHE = {}


def _build_program():
    nc = bacc.Bacc("TRN2", target_bir_lowering=False, debug=False,
                   num_devices=NCORES)
    x_in = nc.dram_tensor("x", [N, D], BF16, kind="ExternalInput").ap()
    negid_in = nc.dram_tensor("negid", [P, P], BF16, kind="ExternalInput").ap()
    md_out = nc.dram_tensor("maxdot", [P, MT], F32, kind="ExternalOutput").ap()

    with tile.TileContext(nc) as tc:
        with (
            tc.tile_pool(name="consts", bufs=1) as const_pool,
            tc.tile_pool(name="xin", bufs=3) as xin_pool,
            tc.tile_pool(name="xnorm", bufs=3) as xn_pool,
            tc.tile_pool(name="stats", bufs=1) as stat_pool,
            tc.tile_pool(name="small", bufs=2) as small_pool,
            tc.tile_pool(name="sq", bufs=2) as sq_pool,
            tc.tile_pool(name="xt", bufs=1) as xt_pool,
            tc.tile_pool(name="xndram", bufs=1, space="DRAM") as dram_pool,
            tc.tile_pool(name="psum", bufs=4, space="PSUM") as psum_pool,
            tc.tile_pool(name="conv", bufs=4) as conv_pool,
            tc.tile_pool(name="accb", bufs=1) as accb_pool,
            tc.tile_pool(name="parts", bufs=1) as part_pool,
            tc.tile_pool(name="outp", bufs=1) as out_pool,
        ):
            negid = const_pool.tile([P, P], BF16)
            nc.sync.dma_start(negid, negid_in)

            ss_all = stat_pool.tile([P, RG * CPG], F32)   # row sum-of-squares
            rn_all = stat_pool.tile([P, RG * CPG], F32)   # 1/row-norm
            xn_dram = dram_pool.tile([N, D], BF16)
            xT = []
            for k in range(KT):
                xTk = xt_pool.tile([P, N], BF16, name=f"xT{k}", tag=f"xT{k}")
                xT.append(xTk)

            # persistent per-mt accumulators across column groups
            accb = []
            parts = []
            for mt in range(MT):
                ab = accb_pool.tile([P, 1024], BF16, name=f"accb{mt}",
                                    tag=f"accb{mt}")
                nc.gpsimd.memset(ab, -3.0)
                accb.append(ab)
                pt = part_pool.tile([P, RG + 1], F32, name=f"parts{mt}",
                                    tag=f"parts{mt}")
                nc.gpsimd.memset(pt, -3.0)
                parts.append(pt)

            x_view = x_in.rearrange("(g s p) d -> g p s d", p=P, s=CPG)
            xn_view = xn_dram.rearrange("(g s p) d -> g p s d", p=P, s=CPG)
            for g in range(RG):
                # ---- load, row norms, scale, store normalized bf16 ----
                xg = xin_pool.tile([P, CPG, D], BF16, tag="xg")
                nc.sync.dma_start(xg, x_view[g])
                for s in range(CPG):
                    c = g * CPG + s
                    sq = sq_pool.tile([P, D], BF16, tag="sq")
                    nc.scalar.activation(sq, xg[:, s], AF.Square,
                                         accum_out=ss_all[:, c:c + 1])
                # rnorm = rsqrt(ss): recip -> sqrt -> 1 Newton step
                sl = ss_all[:, g * CPG:(g + 1) * CPG]
                rn = rn_all[:, g * CPG:(g + 1) * CPG]
                rec = small_pool.tile([P, CPG], F32, tag="rec")
                nc.vector.reciprocal(rec, sl)
                nc.scalar.activation(rn, rec, AF.Sqrt)
                t1 = small_pool.tile([P, CPG], F32, tag="t1")
                nc.vector.tensor_mul(t1, rn, rn)
                nc.vector.tensor_mul(t1, t1, sl)
                nc.vector.tensor_scalar(t1, t1, -0.5, 1.5, OP.mult, OP.add)
                nc.vector.tensor_mul(rn, rn, t1)
                xng = xn_pool.tile([P, CPG, D], BF16, tag="xng")
                for s in range(CPG):
                    c = g * CPG + s
                    nc.vector.tensor_scalar_mul(
                        xng[:, s], xg[:, s], rn_all[:, c:c + 1])
                nc.gpsimd.dma_start(xn_view[g], xng)
                # ---- transpose this group's 1024 columns back in ----
                for k in range(KT):
                    nc.sync.dma_start_transpose(
                        xT[k][:, g * 1024:(g + 1) * 1024],
                        xn_dram[g * 1024:(g + 1) * 1024, k * P:(k + 1) * P])
                # ---- matmuls + reduce for this group's columns ----
                for mt in range(MT):
                    ps = psum_pool.tile([P, 1024], F32, tag="ps")
                    for k in range(KT):
                        for j in range(2):
                            nc.tensor.matmul(
                                ps[:, j * NF:(j + 1) * NF],
                                xT[k][:, mt * P:(mt + 1) * P],
                                xT[k][:, g * 1024 + j * NF:
                                      g * 1024 + (j + 1) * NF],
                                start=(k == 0), stop=(k == KT - 1))
                    if _is_a(g, mt):
                        nc.vector.reduce_max(
                            parts[mt][:, g:g + 1], ps, axis=AX.X)
                    else:
                        cv = conv_pool.tile([P, 1024], BF16, tag="cv")
                        nc.scalar.copy(cv, ps)
                        if g == 0:
                            o = mt * P
                            nc.vector.tensor_add(
                                cv[:, o:o + P], cv[:, o:o + P], negid)
                        nc.vector.tensor_max(accb[mt], accb[mt], cv)

            # ---- finals ----
            md_sb = out_pool.tile([P, MT], F32)
            for mt in range(MT):
                nc.vector.reduce_max(
                    parts[mt][:, RG:RG + 1], accb[mt], axis=AX.X)
                nc.vector.reduce_max(md_sb[:, mt:mt + 1], parts[mt], axis=AX.X)
            nc.sync.dma_start(md_out, md_sb)

    nc.compile()
    return nc


def _get_program():
    if "nc" not in _CACHE:
        _CACHE["nc"] = _build_program()
    return _CACHE["nc"]


def _make_in_maps(student_output: np.ndarray):
    x = np.asarray(student_output, dtype=np.float32)
    assert x.shape == (N, D)
    negid = (MASKVAL * np.eye(P, dtype=np.float32)).astype(ml_dtypes.bfloat16)
    in_maps = []
    for m in range(NCORES):
        xr = np.roll(x, -1024 * m, axis=0).astype(ml_dtypes.bfloat16)
        in_maps.append({"x": xr, "negid": negid})
    return in_maps


def _combine(results) -> np.float32:
    md = np.empty(N, dtype=np.float64)
    for m in range(NCORES):
        blk = np.asarray(results[m]["maxdot"], dtype=np.float64)  # [P, MT]
        md[m * 1024:(m + 1) * 1024] = blk.T.reshape(-1)
    d2 = np.maximum(2.0 - 2.0 * md, 0.0)
    d = np.sqrt(d2)
    loss = -np.mean(np.log(d + 1e-8))
    return np.float32(loss)


def run(student_output: np.ndarray, trace: bool = False):
    nc = _get_program()
    in_maps = _make_in_maps(student_output)
    res = run_bass_kernel_spmd(nc, in_maps, core_ids=list(range(NCORES)),
                               trace=trace)
    return _combine(res.results), res


def kernel(student_output: np.ndarray) -> np.ndarray:
    out, _ = run(student_output,
                 trace=bool(int(os.environ.get("KOLEO_TRACE", "0"))))
    return out


# revision 12
# speedup vs baseline: 1.2739x; 1.0517x over previous
"""KoLeo loss kernel for Trainium2 (8 NeuronCores, Bass/Tile).

Math: reference normalizes rows of student_output [8192, 384], finds each
row's nearest neighbor by cosine similarity (self masked), and returns
  loss = -mean(log(||x_i - x_nn|| + eps)).
For unit vectors ||x_i - x_j||^2 = 2 - 2*dot(x_i, x_j), so only the max
off-diagonal dot per row is needed -- no argmax indices, no gather.

Sharding: core m gets x rotated by m*1024 rows (bf16). It normalizes on
device, transposes via DMA-xbar round trip to get x^T in SBUF, computes its
[1024, 8192] similarity block (bf16 matmul, fp32 PSUM), masks the diagonal
(which the rotation pins to columns [mt*128, mt*128+128) of m-tile mt), and
row-max-reduces split across DVE (direct fp32 reduce from PSUM) and
ACT->bf16-convert + DVE tensor_max (2x mode). Host combines the 8 per-core
[128, 8] max-dot tiles into the scalar loss.
"""

import os
import numpy as np
import ml_dtypes

import concourse.bass as bass
import concourse.tile as tile
from concourse import bacc, mybir
from concourse.bass_utils import run_bass_kernel_spmd

F32 = mybir.dt.float32
BF16 = mybir.dt.bfloat16
AX = mybir.AxisListType
OP = mybir.AluOpType
AF = mybir.ActivationFunctionType

N, D = 8192, 384
P = 128
NCORES = 8
KT = D // P            # 3 contraction tiles
RG = 8                 # row groups of 1024
CPG = 8                # chunks of 128 rows per group
MT = 8                 # m-tiles: own block = first 1024 rotated rows
NF = 512               # matmul moving free dim (one PSUM bank)
NT = N // NF           # 16 n-tiles
MASKVAL = -4.0         # diag additive mask; masked value ~ -3 < any cosine

# Reduce-path per (group, mt) unit (one [128, 1024] PSUM pair):
#   'A' = DVE reduce_max direct from PSUM (fp32, 1x)
#   'B' = ACT copy->bf16 SBUF, DVE tensor_max accumulate (2x)
# group 0 units must be 'B' (diag mask applied on the SBUF copy).
def _is_a(g, mt):
    return g in (2, 4, 6) or (g == 7 and mt < 4)


def _is_gps(g, mt):
    return False  # GpSimd lacks the tensor_tensor max opcode on TRN2

_CACHE = {}


def _build_program():
    nc = bacc.Bacc("TRN2", target_bir_lowering=False, debug=False,
                   num_devices=NCORES)
    x_in = nc.dram_tensor("x", [N, D], BF16, kind="ExternalInput").ap()
    negid_in = nc.dram_tensor("negid", [P, P], BF16, kind="ExternalInput").ap()
    md_out = nc.dram_tensor("maxdot", [P, MT], F32, kind="ExternalOutput").ap()

    with tile.TileContext(nc) as tc:
        with (
            tc.tile_pool(name="consts", bufs=1) as const_pool,
            tc.tile_pool(name="xin", bufs=1) as xin_pool,
            tc.tile_pool(name="xnorm", bufs=3) as xn_pool,
            tc.tile_pool(name="stats", bufs=1) as stat_pool,
            tc.tile_pool(name="small", bufs=2) as small_pool,
            tc.tile_pool(name="sq", bufs=2) as sq_pool,
            tc.tile_pool(name="xt", bufs=1) as xt_pool,
            tc.tile_pool(name="xndram", bufs=1, space="DRAM") as dram_pool,
            tc.tile_pool(name="psum", bufs=4, space="PSUM") as psum_pool,
            tc.tile_pool(name="conv", bufs=4) as conv_pool,
            tc.tile_pool(name="accb", bufs=1) as accb_pool,
            tc.tile_pool(name="parts", bufs=1) as part_pool,
            tc.tile_pool(name="outp", bufs=1) as out_pool,
        ):
            negid = const_pool.tile([P, P], BF16)
            nc.sync.dma_start(negid, negid_in)

            ss_all = stat_pool.tile([P, RG * CPG], F32)   # row sum-of-squares
            rn_all = stat_pool.tile([P, RG * CPG], F32)   # 1/row-norm
            xn_dram = dram_pool.tile([N, D], BF16)
            xT = []
            for k in range(KT):
                xTk = xt_pool.tile([P, N], BF16, name=f"xT{k}", tag=f"xT{k}")
                xT.append(xTk)

            # persistent per-mt accumulators across column groups
            accb = []
            accg = {}
            parts = []
            for mt in range(MT):
                ab = accb_pool.tile([P, 1024], BF16, name=f"accb{mt}",
                                    tag=f"accb{mt}")
                nc.gpsimd.memset(ab, -3.0)
                accb.append(ab)
                pt = part_pool.tile([P, RG + 1], F32, name=f"parts{mt}",
                                    tag=f"parts{mt}")
                nc.gpsimd.memset(pt, -3.0)
                parts.append(pt)

            x_view = x_in.rearrange("(g s p) d -> g p s d", p=P, s=CPG)
            xn_view = xn_dram.rearrange("(g s p) d -> g p s d", p=P, s=CPG)

            xgs = []
            for g in range(RG):
                xg = xin_pool.tile([P, CPG, D], BF16, tag=f"xg{g}",
                                   name=f"xg{g}")
                nc.sync.dma_start(xg, x_view[g])
                xgs.append(xg)

            def phase1(g):
                xg = xgs[g]
                for s in range(CPG):
                    c = g * CPG + s
                    sq = sq_pool.tile([P, D], BF16, tag="sq")
                    nc.scalar.activation(sq, xg[:, s], AF.Square,
                                         accum_out=ss_all[:, c:c + 1])
                # rnorm = rsqrt(ss): recip -> sqrt -> 1 Newton step
                sl = ss_all[:, g * CPG:(g + 1) * CPG]
                rn = rn_all[:, g * CPG:(g + 1) * CPG]
                rec = small_pool.tile([P, CPG], F32, tag="rec")
                nc.vector.reciprocal(rec, sl)
                nc.scalar.activation(rn, rec, AF.Sqrt)
                t1 = small_pool.tile([P, CPG], F32, tag="t1")
                nc.vector.tensor_mul(t1, rn, rn)
                nc.vector.tensor_mul(t1, t1, sl)
                nc.vector.tensor_scalar(t1, t1, -0.5, 1.5, OP.mult, OP.add)
                nc.vector.tensor_mul(rn, rn, t1)
                xng = xn_pool.tile([P, CPG, D], BF16, tag="xng")
                for s in range(CPG):
                    c = g * CPG + s
                    nc.vector.tensor_scalar_mul(
                        xng[:, s], xg[:, s], rn_all[:, c:c + 1])
                nc.gpsimd.dma_start(xn_view[g], xng)
                for k in range(KT):
                    nc.sync.dma_start_transpose(
                        xT[k][:, g * 1024:(g + 1) * 1024],
                        xn_dram[g * 1024:(g + 1) * 1024, k * P:(k + 1) * P])

            def mm_reduce(g):
                for mt in range(MT):
                    ps = psum_pool.tile([P, 1024], F32, tag="ps")
                    for k in range(KT):
                        for j in range(2):
                            nc.tensor.matmul(
                                ps[:, j * NF:(j + 1) * NF],
                                xT[k][:, mt * P:(mt + 1) * P],
                                xT[k][:, g * 1024 + j * NF:
                                      g * 1024 + (j + 1) * NF],
                                start=(k == 0), stop=(k == KT - 1))
                    if _is_a(g, mt):
                        nc.vector.reduce_max(
                            parts[mt][:, g:g + 1], ps, axis=AX.X)
                    else:
                        cv = conv_pool.tile([P, 1024], BF16, tag="cv")
                        nc.scalar.copy(cv, ps)
                        if g == 0:
                            o = mt * P
                            nc.vector.tensor_add(
                                cv[:, o:o + P], cv[:, o:o + P], negid)
                        if _is_gps(g, mt):
                            nc.gpsimd.tensor_max(accg[mt], accg[mt], cv)
                        else:
                            nc.vector.tensor_max(accb[mt], accb[mt], cv)

            # software pipeline: phase-1 of group g+1 outprioritizes the
            # reduce backlog of group g on ACT/DVE
            phase1(0)
            for g in range(1, RG):
                phase1(g)
                mm_reduce(g - 1)
            mm_reduce(RG - 1)

            # ---- finals ----
            md_sb = out_pool.tile([P, MT], F32)
            for mt in range(MT):
                if mt in accg:
                    nc.vector.tensor_max(accb[mt], accb[mt], accg[mt])
                nc.vector.reduce_max(
                    parts[mt][:, RG:RG + 1], accb[mt], axis=AX.X)
                nc.vector.reduce_max(md_sb[:, mt:mt + 1], parts[mt], axis=AX.X)
            nc.sync.dma_start(md_out, md_sb)

    nc.compile()
    return nc


def _get_program():
    if "nc" not in _CACHE:
        _CACHE["nc"] = _build_program()
    return _CACHE["nc"]


def _make_in_maps(student_output: np.ndarray):
    x = np.asarray(student_output, dtype=np.float32)
    assert x.shape == (N, D)
    negid = (MASKVAL * np.eye(P, dtype=np.float32)).astype(ml_dtypes.bfloat16)
    in_maps = []
    for m in range(NCORES):
        xr = np.roll(x, -1024 * m, axis=0).astype(ml_dtypes.bfloat16)
        in_maps.append({"x": xr, "negid": negid})
    return in_maps


def _combine(results) -> np.float32:
    md = np.empty(N, dtype=np.float64)
    for m in range(NCORES):
        blk = np.asarray(results[m]["maxdot"], dtype=np.float64)  # [P, MT]
        md[m * 1024:(m + 1) * 1024] = blk.T.reshape(-1)
    d2 = np.maximum(2.0 - 2.0 * md, 0.0)
    d = np.sqrt(d2)
    loss = -np.mean(np.log(d + 1e-8))
    return np.float32(loss)


def run(student_output: np.ndarray, trace: bool = False):
    nc = _get_program()
    in_maps = _make_in_maps(student_output)
    res = run_bass_kernel_spmd(nc, in_maps, core_ids=list(range(NCORES)),
                               trace=trace)
    return _combine(res.results), res


def kernel(student_output: np.ndarray) -> np.ndarray:
    out, _ = run(student_output,
                 trace=bool(int(os.environ.get("KOLEO_TRACE", "0"))))
    return out
